# revision 9
# baseline (speedup 1.0000x reference)
"""GAT (3-layer, PyG-style) Trainium2 Bass kernel, sharded across 8 NeuronCores.

Sharding: destination-node range partition (graph parallel). Each core owns
N/8 contiguous nodes: it computes h = X_own @ W' for its nodes, AllGathers
h_ext across the 8 cores, then processes all edges whose dst is in its range
(gather h[src] rows via indirect DMA, segment-softmax + weighted scatter-add
via selection-matrix matmuls into PSUM).

kernel(**inputs) takes the FULL inputs and returns the FULL [N, 16] output.
"""

import sys

sys.path.insert(0, "/opt/trn_rl_repo")

import numpy as np

import concourse.bass as bass
import concourse.mybir as mybir
import concourse.tile as tile
from concourse import bacc
from concourse import bass_utils
from concourse.bass import IndirectOffsetOnAxis
from concourse.bass_interp import get_hw_module
from concourse.masks import make_identity

F32 = mybir.dt.float32
I32 = mybir.dt.int32
P = 128


def real_cfg():
    R = 8
    N = 50000
    PER = N // R                      # 6250 nodes per core
    T = (PER + P - 1) // P            # 49 dst tiles per core
    return dict(
        R=R, N=N, PER=PER, T=T, NPAD=T * P,
        F_IN=128, HID=64, HEADS=8, N_CLASSES=16,
        NEG=0.2,
    )


# ---------------------------------------------------------------------------
# Host-side preprocessing
# ---------------------------------------------------------------------------

def host_prepare(inputs, cfg):
    """Build per-core in_maps (numpy) from full inputs. Returns (in_maps, B)."""
    R, N, PER, T, NPAD = cfg["R"], cfg["N"], cfg["PER"], cfg["T"], cfg["NPAD"]
    F_IN, HID, HEADS, NCLS = cfg["F_IN"], cfg["HID"], cfg["HEADS"], cfg["N_CLASSES"]
    HC = HID * HEADS

    x = np.asarray(inputs["x"], np.float32)
    ei = np.asarray(inputs["edge_index"])
    src = np.concatenate([ei[0], np.arange(N, dtype=ei.dtype)]).astype(np.int64)
    dst = np.concatenate([ei[1], np.arange(N, dtype=ei.dtype)]).astype(np.int64)

    core = dst // PER
    dloc = (dst - core * PER).astype(np.int64)

    # per (core, tile) edge counts -> global max block count B
    tile_of = dloc // P
    counts = np.zeros((R, T), np.int64)
    np.add.at(counts, (core, tile_of), 1)
    B = int(np.ceil(counts.max() / P))

    # gather row index of a global node id inside the allgathered h_ext
    gsrc_all = (src // PER) * NPAD + (src % PER)

    eidx_src = np.zeros((R, T, P, B), np.int32)
    eidx_dst = np.zeros((R, T, P, B), np.int32)
    dlc = np.full((R, T, P, B), -1.0, np.float32)

    order = np.lexsort((dloc, core))  # sort by core then dst_local
    src_s = gsrc_all[order]
    dst_s = dloc[order]
    core_s = core[order]
    tile_s = tile_of[order]
    # position of each edge within its (core, tile) group
    grp = (core_s * T + tile_s)
    # edges are sorted by grp; rank within group:
    grp_start = np.searchsorted(grp, np.arange(R * T), side="left")
    pos = np.arange(len(grp)) - grp_start[grp]
    pp = (pos % P).astype(np.int64)
    bb = (pos // P).astype(np.int64)
    eidx_src[core_s, tile_s, pp, bb] = src_s.astype(np.int32)
    eidx_dst[core_s, tile_s, pp, bb] = dst_s.astype(np.int32)
    dlc[core_s, tile_s, pp, bb] = (dst_s - tile_s * P).astype(np.float32)

    # weight assembly: W'[f, :] = [W | W.a_src | W.a_dst] (+ pad for layer 2)
    def wext(W, a_s, a_d, ncols):
        Fin = W.shape[0]
        H, C = a_s.shape
        Wr = W.reshape(Fin, H, C)
        We = np.zeros((Fin, ncols), np.float32)
        We[:, : H * C] = W
        We[:, H * C : H * C + H] = np.einsum("fhc,hc->fh", Wr, a_s)
        We[:, H * C + H : H * C + 2 * H] = np.einsum("fhc,hc->fh", Wr, a_d)
        return We

    ROW = HC + 2 * HEADS          # 528
    ROW2 = 24                     # 16 + 1 + 1 + pad
    W0e = wext(np.asarray(inputs["W0"], np.float32),
               np.asarray(inputs["a_s0"], np.float32),
               np.asarray(inputs["a_d0"], np.float32), ROW)
    W1e = wext(np.asarray(inputs["W1"], np.float32),
               np.asarray(inputs["a_s1"], np.float32),
               np.asarray(inputs["a_d1"], np.float32), ROW)
    W2e = wext(np.asarray(inputs["W2"], np.float32),
               np.asarray(inputs["a_s2"], np.float32),
               np.asarray(inputs["a_d2"], np.float32), ROW2)

    def bext(b, ncols):
        be = np.zeros((1, ncols), np.float32)
        be[0, : b.shape[0]] = b
        return np.ascontiguousarray(np.broadcast_to(be, (P, ncols)))

    b0e = bext(np.asarray(inputs["b0"], np.float32), ROW)
    b1e = bext(np.asarray(inputs["b1"], np.float32), ROW)
    b2e = bext(np.asarray(inputs["b2"], np.float32), ROW2)

    # layer-1/2 weights reshaped to [128, KC, ROW]
    W1e_r = W1e.reshape(4, P, ROW).transpose(1, 0, 2).copy()
    W2e_r = W2e.reshape(4, P, ROW2).transpose(1, 0, 2).copy()

    in_maps = []
    for r in range(R):
        xt0 = np.ascontiguousarray(x[r * PER : (r + 1) * PER].T)  # [F_IN, PER]
        in_maps.append({
            "xt0": xt0,
            "w0e": W0e, "w1e": W1e_r, "w2e": W2e_r,
            "b0e": b0e, "b1e": b1e, "b2e": b2e,
            "eidx_src": eidx_src[r], "eidx_dst": eidx_dst[r], "dlc": dlc[r],
        })
    return in_maps, B


# ---------------------------------------------------------------------------
# Device program
# ---------------------------------------------------------------------------

def build_gat_nc(cfg, B):
    R, PER, T, NPAD = cfg["R"], cfg["PER"], cfg["T"], cfg["NPAD"]
    F_IN, HID, HEADS, NCLS = cfg["F_IN"], cfg["HID"], cfg["HEADS"], cfg["N_CLASSES"]
    NEG = cfg["NEG"]
    HC = HID * HEADS
    ROW = HC + 2 * HEADS
    ROW2 = 24

    nc = bacc.Bacc("TRN2", target_bir_lowering=False, debug=False,
                   num_devices=R)

    xt0_d = nc.dram_tensor("xt0", [F_IN, PER], F32, kind="ExternalInput")
    w0e_d = nc.dram_tensor("w0e", [F_IN, ROW], F32, kind="ExternalInput")
    w1e_d = nc.dram_tensor("w1e", [P, 4, ROW], F32, kind="ExternalInput")
    w2e_d = nc.dram_tensor("w2e", [P, 4, ROW2], F32, kind="ExternalInput")
    b0e_d = nc.dram_tensor("b0e", [P, ROW], F32, kind="ExternalInput")
    b1e_d = nc.dram_tensor("b1e", [P, ROW], F32, kind="ExternalInput")
    b2e_d = nc.dram_tensor("b2e", [P, ROW2], F32, kind="ExternalInput")
    esrc_d = nc.dram_tensor("eidx_src", [T, P, B], I32, kind="ExternalInput")
    edst_d = nc.dram_tensor("eidx_dst", [T, P, B], I32, kind="ExternalInput")
    dlc_d = nc.dram_tensor("dlc", [T, P, B], F32, kind="ExternalInput")
    out_d = nc.dram_tensor("out", [PER, NCLS], F32, kind="ExternalOutput")

    rg = [list(range(R))]

    with tile.TileContext(nc) as tc:
        with (
            tc.tile_pool(name="pers", bufs=1) as pers,
            tc.tile_pool(name="sb", bufs=2) as sb,
            tc.tile_pool(name="sb3", bufs=3) as sb3,
            tc.tile_pool(name="ps", bufs=2, space="PSUM") as ps,
            tc.tile_pool(name="ps1", bufs=1, space="PSUM") as ps1,
            tc.tile_pool(name="dram", bufs=1, space="DRAM") as dram,
        ):
            # ---- persistent tiles ----
            Xt = pers.tile([P, 4, NPAD], F32)          # feature-major X (own nodes)
            iota_i = pers.tile([P, P], I32)
            iota_row = pers.tile([P, P], F32)
            ident = pers.tile([P, P], F32)
            nc.gpsimd.iota(iota_i[:], pattern=[[1, P]], base=0, channel_multiplier=0)
            nc.vector.tensor_copy(iota_row[:], iota_i[:])
            make_identity(nc, ident[:])

            w0_sb = pers.tile([P, 1, ROW], F32)
            w1_sb = pers.tile([P, 4, ROW], F32)
            w2_sb = pers.tile([P, 4, ROW2], F32)
            b0_sb = pers.tile([P, ROW], F32)
            b1_sb = pers.tile([P, ROW], F32)
            b2_sb = pers.tile([P, ROW2], F32)
            nc.sync.dma_start(w0_sb[:, 0, :], w0e_d[:, :])
            nc.sync.dma_start(w1_sb[:], w1e_d[:, :, :])
            nc.sync.dma_start(w2_sb[:], w2e_d[:, :, :])
            nc.sync.dma_start(b0_sb[:], b0e_d[:, :])
            nc.sync.dma_start(b1_sb[:], b1e_d[:, :])
            nc.sync.dma_start(b2_sb[:], b2e_d[:, :])

            # layer 0 X^T from host
            nc.sync.dma_start(Xt[:, 0, :PER], xt0_d[:, :])

            # ---- internal DRAM ----
            hxl = [
                dram.tile([NPAD, ROW], F32, name="hxl0"),
                dram.tile([NPAD, ROW], F32, name="hxl1"),
                dram.tile([NPAD, ROW2], F32, name="hxl2"),
            ]
            hxf = [
                dram.tile([R * NPAD, ROW], F32, addr_space="Shared", name="hxf0"),
                dram.tile([R * NPAD, ROW], F32, addr_space="Shared", name="hxf1"),
                dram.tile([R * NPAD, ROW2], F32, addr_space="Shared", name="hxf2"),
            ]

            for L in range(3):
                row = ROW if L < 2 else ROW2
                KC = 1 if L == 0 else 4
                nH = HEADS if L < 2 else 1
                ncols = HC if L < 2 else NCLS
                W_sb = [w0_sb, w1_sb, w2_sb][L]
                b_sb = [b0_sb, b1_sb, b2_sb][L]
                alow = ncols            # first al_s column
                adoff = ncols + nH      # first al_d column

                # ---------- h_ext = X_own @ W' + b' ----------
                for nt in range(T):
                    ph = ps1.tile([P, row], F32, tag="ph")
                    for kc in range(KC):
                        nc.tensor.matmul(
                            ph[:, 0:512] if row > 512 else ph[:, 0:row],
                            lhsT=Xt[:, kc, nt * P : (nt + 1) * P],
                            rhs=W_sb[:, kc, 0:512] if row > 512 else W_sb[:, kc, 0:row],
                            start=(kc == 0), stop=(kc == KC - 1),
                        )
                    if row > 512:
                        for kc in range(KC):
                            nc.tensor.matmul(
                                ph[:, 512:row],
                                lhsT=Xt[:, kc, nt * P : (nt + 1) * P],
                                rhs=W_sb[:, kc, 512:row],
                                start=(kc == 0), stop=(kc == KC - 1),
                            )
                    hsb = sb3.tile([P, row], F32, tag="hsb")
                    nc.vector.tensor_tensor(
                        hsb[:], ph[:], b_sb[:],
                        mybir.AluOpType.add,
                    )
                    nc.sync.dma_start(hxl[L][nt * P : (nt + 1) * P, :], hsb[:])

                # ---------- AllGather ----------
                nc.gpsimd.collective_compute(
                    "AllGather", mybir.AluOpType.bypass,
                    replica_groups=rg,
                    ins=[hxl[L][:, :]],
                    outs=[hxf[L][:, :]],
                )

                # ---------- edge aggregation per dst tile ----------
                for t in range(T):
                    idxs = sb.tile([P, B], I32, tag="idxs")
                    idxd = sb.tile([P, B], I32, tag="idxd")
                    dlct = sb.tile([P, B], F32, tag="dlct")
                    nc.sync.dma_start(idxs[:], esrc_d[t, :, :])
                    nc.sync.dma_start(idxd[:], edst_d[t, :, :])
                    nc.sync.dma_start(dlct[:], dlc_d[t, :, :])

                    g = sb.tile([P, B, row], F32, tag="g")
                    adem = sb.tile([P, B, nH], F32, tag="adem")
                    for b in range(B):
                        nc.gpsimd.indirect_dma_start(
                            out=g[:, b, :], out_offset=None,
                            in_=hxf[L][:, :],
                            in_offset=IndirectOffsetOnAxis(ap=idxs[:, b : b + 1], axis=0),
                        )
                        nc.gpsimd.indirect_dma_start(
                            out=adem[:, b, :], out_offset=None,
                            in_=hxl[L][:, :],
                            in_offset=IndirectOffsetOnAxis(ap=idxd[:, b : b + 1], axis=0),
                            element_offset=adoff,
                        )

                    # logits = al_s[src] + al_d[dst] ; lrelu ; exp
                    logits = sb.tile([P, B * nH], F32, tag="logits")
                    nc.vector.tensor_tensor(
                        logits[:].rearrange("p (b h) -> p b h", b=B),
                        g[:, :, alow : alow + nH],
                        adem[:],
                        mybir.AluOpType.add,
                    )
                    lr = sb.tile([P, B * nH], F32, tag="lr")
                    nc.vector.tensor_scalar_mul(lr[:], logits[:], NEG)
                    nc.vector.tensor_tensor(lr[:], lr[:], logits[:],
                                            mybir.AluOpType.max)
                    w = sb.tile([P, B * nH], F32, tag="w")
                    nc.scalar.activation(w[:], lr[:],
                                         mybir.ActivationFunctionType.Exp)

                    # weight gathered rows in place (al columns untouched)
                    nc.vector.tensor_tensor(
                        g[:, :, 0:ncols].rearrange("p b (h c) -> p b h c", h=nH),
                        g[:, :, 0:ncols].rearrange("p b (h c) -> p b h c", h=nH),
                        w[:].rearrange("p (b h) -> p b h", b=B)
                            .unsqueeze(3)
                            .to_broadcast([P, B, nH, ncols // nH]),
                        mybir.AluOpType.mult,
                    )

                    po = ps.tile([P, ncols], F32, tag="po")
                    pd = ps.tile([P, nH], F32, tag="pd")
                    for b in range(B):
                        S_b = sb.tile([P, P], F32, tag=f"S{b}")
                        nc.vector.tensor_tensor(
                            S_b[:],
                            dlct[:, b : b + 1].to_broadcast([P, P]),
                            iota_row[:],
                            mybir.AluOpType.is_equal,
                        )
                        nc.tensor.matmul(po[:], lhsT=S_b[:], rhs=g[:, b, 0:ncols],
                                         start=(b == 0), stop=(b == B - 1))
                        nc.tensor.matmul(pd[:], lhsT=S_b[:],
                                         rhs=w[:, b * nH : (b + 1) * nH],
                                         start=(b == 0), stop=(b == B - 1))

                    den = sb.tile([P, nH], F32, tag="den")
                    nc.vector.tensor_scalar_max(den[:], pd[:], 1e-30)
                    rden = sb.tile([P, nH], F32, tag="rden")
                    nc.vector.reciprocal(rden[:], den[:])
                    xn = sb.tile([P, ncols], F32, tag="xn")
                    nc.vector.tensor_tensor(
                        xn[:].rearrange("p (h c) -> p h c", h=nH),
                        po[:].rearrange("p (h c) -> p h c", h=nH),
                        rden[:].unsqueeze(2).to_broadcast([P, nH, ncols // nH]),
                        mybir.AluOpType.mult,
                    )

                    if L < 2:
                        # ELU: xe = relu(x) + exp(min(x,0)) - 1
                        m = sb.tile([P, ncols], F32, tag="m")
                        nc.vector.tensor_scalar_min(m[:], xn[:], 0.0)
                        em = sb.tile([P, ncols], F32, tag="em")
                        nc.scalar.activation(em[:], m[:],
                                             mybir.ActivationFunctionType.Exp)
                        xe = sb.tile([P, ncols], F32, tag="xe")
                        nc.vector.tensor_scalar_max(xe[:], xn[:], 0.0)
                        nc.vector.tensor_tensor(xe[:], xe[:], em[:],
                                                mybir.AluOpType.add)
                        nc.vector.tensor_scalar_add(xe[:], xe[:], -1.0)
                        for c4 in range(ncols // P):
                            pt = ps.tile([P, P], F32, tag="pt")
                            nc.tensor.transpose(
                                pt[:], xe[:, c4 * P : (c4 + 1) * P], ident[:])
                            nc.scalar.copy(Xt[:, c4, t * P : (t + 1) * P], pt[:])
                    else:
                        rows = min(P, PER - t * P)
                        nc.sync.dma_start(out_d[t * P : t * P + rows, :],
                                          xn[:rows, 0:NCLS])

    nc.compile()
    nc.m = get_hw_module(nc.m)
    return nc


# ---------------------------------------------------------------------------
# Entry point
# ---------------------------------------------------------------------------

_CACHE = {}


def _get_nc(cfg, B):
    key = (tuple(sorted(cfg.items())), B)
    if key not in _CACHE:
        _CACHE[key] = build_gat_nc(cfg, B)
    return _CACHE[key]


def run(inputs, trace=False):
    cfg = real_cfg()
    in_maps, B = host_prepare(inputs, cfg)
    nc = _get_nc(cfg, B)
    res = bass_utils.run_bass_kernel_spmd(
        nc, in_maps, core_ids=list(range(cfg["R"])), trace=trace)
    out = np.concatenate([res.results[r]["out"] for r in range(cfg["R"])], axis=0)
    return out[: cfg["N"]], res


def kernel(**inputs) -> np.ndarray:
    out, _ = run(inputs, trace=False)
    return out.astype(np.float32)
